# revision 3
# baseline (speedup 1.0000x reference)
"""Self-contained Trainium2 Bass kernel for nn_Attention_395136991961.

Dense multi-head attention (B=8, N=1024, C=1024, H=16, D=64) with RoPE,
full materialized softmax, and output projection.

Sharding: data-parallel over batch B across the 8 NeuronCores (one batch
element per core, weights replicated, no collectives).

Device-side layout strategy (per core, all matmuls in float32r):
  Phase A  qkv = x @ qkv_w.T computed in natural [token, j] layout
           (lhsT = x^T resident in SBUF, rhs = qkv_w^T streamed from HBM),
           RoPE applied in natural layout (free-dim rotate-half), then q,k
           PE-transposed into [d, token] layout. v kept natural with an
           appended ones column (softmax denominator comes free from the
           augmented matmul).
  Phase B  per (head, m-chunk): S^T = k'^T.T @ q'^T -> exp(scale*S^T) -> P^T;
           O^T (+ sums row) = Vaug.T @ P^T accumulated over token tiles;
           normalize via reciprocal + gpsimd partition broadcast.
  Phase C  y = O @ proj_w.T + b from the packed O^T tiles.
"""

import sys

if "/opt/trn_rl_repo" not in sys.path:
    sys.path.insert(0, "/opt/trn_rl_repo")

import numpy as np

import concourse.tile as tile
import concourse.mybir as mybir
from concourse import bacc
from concourse.bass_utils import run_bass_kernel_spmd

F32 = mybir.dt.float32
F32R = mybir.dt.float32r
AF = mybir.ActivationFunctionType
OP = mybir.AluOpType

N_CORES = 8
C = 1024
H = 16
D = 64
HD2 = D // 2  # rotate-half split
SCALE = float(D) ** -0.5

PROFILE = False
LAST_EXEC_NS = None
_CACHE = {}


def build(n_tok):
    ntile = n_tok // 128          # token tiles
    mch = 512 if n_tok >= 512 else n_tok
    nmch = n_tok // mch           # m chunks per head
    nct = C // 128                # contraction tiles (8)
    njc = (3 * C) // 512          # qkv j-chunks (6)
    nec = C // 512                # proj e-chunks (2)

    nc = bacc.Bacc("TRN2", target_bir_lowering=False, debug=False, num_devices=1)

    xT = nc.dram_tensor("xT", [C, n_tok], F32, kind="ExternalInput").ap()
    wT = nc.dram_tensor("wT", [C, 3 * C], F32, kind="ExternalInput").ap()
    pwT = nc.dram_tensor("pwT", [C, C], F32, kind="ExternalInput").ap()
    pbias = nc.dram_tensor("pbias", [1, C], F32, kind="ExternalInput").ap()
    cosN = nc.dram_tensor("cosN", [n_tok, D], F32, kind="ExternalInput").ap()
    sinA = nc.dram_tensor("sinA", [n_tok, D], F32, kind="ExternalInput").ap()
    identin = nc.dram_tensor("identin", [128, 128], F32, kind="ExternalInput").ap()
    vinit = nc.dram_tensor("vinit", [128, H * (D + 1)], F32, kind="ExternalInput").ap()
    y = nc.dram_tensor("y", [n_tok, C], F32, kind="ExternalOutput").ap()

    xT_t = xT.rearrange("(t p) n -> p t n", p=128)
    wT_t = wT.rearrange("(t p) j -> p t j", p=128)
    pwT_t = pwT.rearrange("(t p) e -> p t e", p=128)
    cos_t = cosN.rearrange("(t p) d -> p t d", p=128)
    sin_t = sinA.rearrange("(t p) d -> p t d", p=128)

    with tile.TileContext(nc) as tc:
        with tc.tile_pool(name="persist", bufs=1) as pp:
            # ---------------- persistent tiles ----------------
            cos_sb = pp.tile([128, ntile, D], F32, tag="cos")
            nc.sync.dma_start(cos_sb[:], cos_t)
            sin_sb = pp.tile([128, ntile, D], F32, tag="sin")
            nc.sync.dma_start(sin_sb[:], sin_t)
            ident = pp.tile([128, 128], F32, tag="ident")
            nc.sync.dma_start(ident[:], identin[:])
            pb_sb = pp.tile([1, C], F32, tag="pb")
            nc.sync.dma_start(pb_sb[:], pbias[:])
            bias_b = pp.tile([128, C], F32, tag="biasb")
            nc.gpsimd.partition_broadcast(bias_b[:], pb_sb[0:1, :])

            qT_sb = pp.tile([128, H // 2, n_tok], F32R, tag="qT")
            kT_sb = pp.tile([128, H // 2, n_tok], F32R, tag="kT")
            v_sb = pp.tile([128, ntile, H, D + 1], F32R, tag="v")
            for t in range(ntile):
                nc.sync.dma_start(
                    v_sb[:, t, :, :].rearrange("p h d -> p (h d)"),
                    vinit[:].bitcast(F32R),
                )

            # ---------------- Phase A: qkv + RoPE + transposes -------------
            with (
                tc.tile_pool(name="xtp", bufs=1) as xtp,
                tc.tile_pool(name="wstream", bufs=2) as wsp,
                tc.tile_pool(name="ropetmp", bufs=2) as rtp,
                tc.tile_pool(name="psA", bufs=3, space="PSUM") as psA,
                tc.tile_pool(name="psT", bufs=2, space="PSUM") as psT,
            ):
                xT_sb = xtp.tile([128, nct, n_tok], F32R, tag="xT")
                nc.sync.dma_start(xT_sb[:], xT_t.bitcast(F32R))

                for jc in range(njc):
                    wchunk = wsp.tile([128, nct, 512], F32R, tag="w")
                    nc.sync.dma_start(
                        wchunk[:],
                        wT_t[:, :, jc * 512 : (jc + 1) * 512].bitcast(F32R),
                    )
                    for t in range(ntile):
                        pq = psA.tile([128, 512], F32, tag="pq")
                        for ct in range(nct):
                            nc.tensor.matmul(
                                pq[:],
                                xT_sb[:, ct, t * 128 : (t + 1) * 128],
                                wchunk[:, ct, :],
                                start=(ct == 0),
                                stop=(ct == nct - 1),
                            )
                        if jc < 4:
                            # q (jc 0,1) or k (jc 2,3): RoPE + transpose
                            dstT = qT_sb if jc < 2 else kT_sb
                            half = jc % 2  # which 8-head half of q/k
                            pq3 = pq[:].rearrange("p (h d) -> p h d", d=D)
                            cos3 = (
                                cos_sb[:, t, :]
                                .rearrange("p (o d) -> p o d", d=D)
                                .to_broadcast([128, 8, D])
                            )
                            sinlo = (
                                sin_sb[:, t, 0:HD2]
                                .rearrange("p (o d) -> p o d", d=HD2)
                                .to_broadcast([128, 8, HD2])
                            )
                            sinhi = (
                                sin_sb[:, t, HD2:D]
                                .rearrange("p (o d) -> p o d", d=HD2)
                                .to_broadcast([128, 8, HD2])
                            )
                            tmp = rtp.tile([128, 512], F32, tag="ropet")
                            tmp3 = tmp[:].rearrange("p (h d) -> p h d", d=D)
                            nc.vector.tensor_tensor(
                                out=tmp3[:, :, 0:HD2],
                                in0=pq3[:, :, HD2:D],
                                in1=sinlo,
                                op=OP.mult,
                            )
                            nc.vector.tensor_tensor(
                                out=tmp3[:, :, HD2:D],
                                in0=pq3[:, :, 0:HD2],
                                in1=sinhi,
                                op=OP.mult,
                            )
                            u = rtp.tile([128, 512], F32, tag="ropeu")
                            nc.vector.tensor_tensor(
                                out=u[:].rearrange("p (h d) -> p h d", d=D),
                                in0=pq3,
                                in1=cos3,
                                op=OP.mult,
                            )
                            qh = rtp.tile([128, 512], F32, tag="ropeq")
                            nc.vector.tensor_tensor(
                                out=qh[:], in0=u[:], in1=tmp[:], op=OP.add
                            )
                            for jb in range(4):
                                pt = psT.tile([128, 128], F32, tag="pt")
                                nc.tensor.transpose(
                                    pt[:],
                                    qh[:, jb * 128 : (jb + 1) * 128],
                                    ident[:],
                                )
                                nc.scalar.copy(
                                    dstT[
                                        :,
                                        half * 4 + jb,
                                        t * 128 : (t + 1) * 128,
                                    ],
                                    pt[:],
                                )
                        else:
                            # v (jc 4,5): copy into augmented layout
                            hb = (jc - 4) * 8
                            nc.scalar.copy(
                                v_sb[:, t, hb : hb + 8, 0:D],
                                pq[:].rearrange("p (h d) -> p h d", d=D),
                            )

            # ------------- Phase B + C (oT spans both) ----------------------
            with tc.tile_pool(name="otp", bufs=1) as otp:
                oT_sb = otp.tile([128, nct, n_tok], F32R, tag="oT")

                with (
                    tc.tile_pool(name="ptpool", bufs=2) as ptp,
                    tc.tile_pool(name="nrm", bufs=2) as nrm,
                    tc.tile_pool(name="psB", bufs=3, space="PSUM") as psB,
                    tc.tile_pool(name="psO", bufs=2, space="PSUM") as psO,
                ):
                    for h in range(H):
                        jt = h // 2
                        rb = (h % 2) * 64
                        for mc in range(nmch):
                            ms = mc * mch
                            pT = ptp.tile([128, ntile, mch], F32R, tag="pT")
                            for t in range(ntile):
                                ps_ = psB.tile([128, mch], F32, tag="ps")
                                nc.tensor.matmul(
                                    ps_[:],
                                    kT_sb[
                                        rb : rb + 64, jt, t * 128 : (t + 1) * 128
                                    ],
                                    qT_sb[rb : rb + 64, jt, ms : ms + mch],
                                    start=True,
                                    stop=True,
                                )
                                nc.scalar.activation(
                                    pT[:, t, :], ps_[:], AF.Exp, scale=SCALE
                                )
                            po = psO.tile([65, mch], F32, tag="po")
                            for t in range(ntile):
                                nc.tensor.matmul(
                                    po[:],
                                    v_sb[:, t, h, :],
                                    pT[:, t, :],
                                    start=(t == 0),
                                    stop=(t == ntile - 1),
                                )
                            # normalization: sums live in row 64 of po
                            ssb = nrm.tile([128, mch], F32, tag="ssb")
                            nc.scalar.copy(ssb[64:65, :], po[64:65, :])
                            s0 = nrm.tile([1, mch], F32, tag="s0")
                            nc.sync.dma_start(s0[:], ssb[64:65, :])
                            rs0 = nrm.tile([1, mch], F32, tag="rs0")
                            nc.vector.reciprocal_approx_fast(
                                out=rs0[:], in_=s0[:]
                            )
                            rb_sb = nrm.tile([64, mch], F32, tag="rb")
                            nc.gpsimd.partition_broadcast(rb_sb[:], rs0[0:1, :])
                            tmpo = nrm.tile([64, mch], F32R, tag="tmpo")
                            nc.vector.tensor_tensor(
                                out=tmpo[:],
                                in0=po[0:64, :],
                                in1=rb_sb[:],
                                op=OP.mult,
                            )
                            nc.sync.dma_start(
                                oT_sb[rb : rb + 64, jt, ms : ms + mch], tmpo[:]
                            )

                # ---------------- Phase C: output projection ----------------
                with (
                    tc.tile_pool(name="pwp", bufs=2) as pwp,
                    tc.tile_pool(name="ypool", bufs=2) as yp,
                    tc.tile_pool(name="psC", bufs=3, space="PSUM") as psC,
                ):
                    for ec in range(nec):
                        pwc = pwp.tile([128, nct, 512], F32R, tag="pw")
                        nc.sync.dma_start(
                            pwc[:],
                            pwT_t[:, :, ec * 512 : (ec + 1) * 512].bitcast(F32R),
                        )
                        for t in range(ntile):
                            py = psC.tile([128, 512], F32, tag="py")
                            for ft in range(nct):
                                nc.tensor.matmul(
                                    py[:],
                                    oT_sb[:, ft, t * 128 : (t + 1) * 128],
                                    pwc[:, ft, :],
                                    start=(ft == 0),
                                    stop=(ft == nct - 1),
                                )
                            ysb = yp.tile([128, 512], F32, tag="y")
                            nc.vector.tensor_tensor(
                                out=ysb[:],
                                in0=py[:],
                                in1=bias_b[:, ec * 512 : (ec + 1) * 512],
                                op=OP.add,
                            )
                            nc.sync.dma_start(
                                y[
                                    t * 128 : (t + 1) * 128,
                                    ec * 512 : (ec + 1) * 512,
                                ],
                                ysb[:],
                            )

    nc.compile()
    return nc


def _host_inputs(x, rope_freqs, qkv_w, proj_w, proj_b):
    x = np.asarray(x, dtype=np.float32)
    rope_freqs = np.asarray(rope_freqs, dtype=np.float32)
    qkv_w = np.asarray(qkv_w, dtype=np.float32)
    proj_w = np.asarray(proj_w, dtype=np.float32)
    proj_b = np.asarray(proj_b, dtype=np.float32)

    B, n_tok, _ = x.shape
    wTh = np.ascontiguousarray(qkv_w.T)
    pwTh = np.ascontiguousarray(proj_w.T)
    freqs = rope_freqs[0, :, 0, :]  # [N, D]
    cosh = np.cos(freqs).astype(np.float32)
    sinh = np.sin(freqs).astype(np.float32)
    sinAh = np.concatenate([-sinh[:, :HD2], sinh[:, HD2:]], axis=1)
    sinAh = np.ascontiguousarray(sinAh)
    identh = np.eye(128, dtype=np.float32)
    vinith = np.zeros((128, H, D + 1), np.float32)
    vinith[:, :, D] = 1.0
    vinith = vinith.reshape(128, H * (D + 1))
    pbh = np.ascontiguousarray(proj_b.reshape(1, C))

    in_maps = []
    for b in range(B):
        in_maps.append(
            {
                "xT": np.ascontiguousarray(x[b].T),
                "wT": wTh,
                "pwT": pwTh,
                "pbias": pbh,
                "cosN": cosh,
                "sinA": sinAh,
                "identin": identh,
                "vinit": vinith,
            }
        )
    return in_maps, n_tok


def kernel(x, rope_freqs, qkv_w, proj_w, proj_b):
    global LAST_EXEC_NS
    in_maps, n_tok = _host_inputs(x, rope_freqs, qkv_w, proj_w, proj_b)
    key = ("nc", n_tok)
    if key not in _CACHE:
        _CACHE[key] = build(n_tok)
    nc = _CACHE[key]

    trace = False
    if PROFILE:
        try:
            import profshim

            profshim.install()
            trace = True
        except Exception:
            trace = False

    res = run_bass_kernel_spmd(
        nc, in_maps, list(range(len(in_maps))), trace=trace
    )
    LAST_EXEC_NS = res.exec_time_ns
    out = np.stack([res.results[b]["y"] for b in range(len(in_maps))], axis=0)
    return out
